# revision 14
# baseline (speedup 1.0000x reference)
"""GP marginal log-likelihood kernel for Trainium2 (Bass/Tile).

Computes -0.5 * y^T A^-1 y - 0.5 * logdet(A) for A = K + sigma^2 I where
K is the RBF covariance on the integer grid 0..T-1 (T=8192).

A is symmetric positive-definite *Toeplitz* and effectively *banded*
(entries vanish below f32 eps for |i-j| > 255 at lengthscale 32), and is
well conditioned: eig(A) in [sigma^2, sigma^2 + v*sum_d k(d)] (~[1, 81.2]).
This kernel exploits that structure instead of a dense 8192^3 factorization:

  * quad = y^T A^-1 y: x = p(A) y where p is a least-squares-optimal
    polynomial fitted on the host (from the hyperparameters alone) to the
    model spectrum of A (the symbol samples), expressed in the Chebyshev
    *second-kind* basis U_m so the device recurrence is the uniform
    three-term form with no special first step:
        q_0 = y,  q_{m+1} = (2As) q_m - q_{m-1}  (q_{-1} = 0),
        x = sum_m gamma_m q_m,
    where each (2As) q is a block-tridiagonal matvec: 3 tensor-engine
    matmuls with 128x128 stationary band blocks (the |i-j| in [129,255]
    tail that falls outside +-1 block reach is ~3e-4 and verified
    numerically to not matter at the required tolerance).  The recurrence
    runs in a *hardware For_i loop*, one step per trip: the q_m live in
    one big SBUF tile as 66-column padded slots addressed by the loop
    variable, and gamma_m comes from a loop-indexed scalar AP.  The
    second-order functional quad = x^T (2y - A x) (with A x recovered
    from the same 2As operator: A x = ((2As)x - sh2 x)/sc2) makes the
    final error quadratic in the solver error (~5e-6 relative, degree 18).
  * logdet via the strong Szego limit theorem:
        logdet A = T*c_0 + sum_{k>=1} k*c_k^2,   c_k = Fourier coeffs of
    log f, f = the symbol of A.  T*c_0 is computed on device as the
    periodic-trapezoid mean of log f over a symmetric 512-point grid in
    [-pi, pi) (spectrally accurate; single Gaussian image suffices there).
    The small correction sum_k k*c_k^2 (~1.8 here) depends only on the
    scalar hyperparameters and is folded into the final combine constant
    on the host, like the polynomial coefficient schedule.

Everything data(y)-dependent runs on the device.  The host computes only
the iteration coefficient schedule and scalar constants from the scalar
hyperparameters (sigma^2, lengthscale, variance).  All 8 cores run the
same program on replicated inputs (the answer is a single scalar; core 0's
result is gathered).
"""

import math

import numpy as np

T = 8192
P = 128  # partitions
NBLK = T // P  # 64 column blocks
SW = NBLK + 2  # 66: one pad column each side of a 64-block slot
N_GRID = 512  # Szego c0 quadrature grid (symmetric, full circle)
NGC = N_GRID // P  # 4 grid columns
N_DEG = 19  # number of polynomial coefficients (highest index 18)
NTRIP = N_DEG - 1  # hardware-loop trips, one recurrence step each
NSLOT = N_DEG + 3  # q_{-1}(=0), q_0..q_{N_DEG}, x accumulator
GW = 32  # gamma table width

_prog_cache = {}


def _ls_poly_U(sig2, ell, var, n_deg):
    """Host-side schedule: LS-optimal solve polynomial in the U basis.

    Fits p(lam) = sum_m gamma_m U_m(s(lam)) minimizing
    sum_j (1 - lam_j p(lam_j))^2 / lam_j over the model spectrum
    lam_j = f(2 pi j / T) (symbol samples, the asymptotic eigenvalue
    distribution of A).  Returns (gamma, lo, hi)."""
    th = np.linspace(0.0, np.pi, T // 2 + 1)
    lam = sig2 + var * ell * math.sqrt(2.0 * math.pi) * (
        np.exp(-((ell * th) ** 2) / 2.0)
        + np.exp(-((ell * (th - 2 * math.pi)) ** 2) / 2.0)
    )
    lo, hi = float(lam.min()), float(lam.max())
    xs = (2.0 * lam - (hi + lo)) / (hi - lo)
    V = np.zeros((lam.size, n_deg))
    V[:, 0] = 1.0
    if n_deg > 1:
        V[:, 1] = 2.0 * xs  # U_1 = 2x
    for m in range(2, n_deg):
        V[:, m] = 2.0 * xs * V[:, m - 1] - V[:, m - 2]
    w = 1.0 / lam
    Aw = V * (lam * np.sqrt(w))[:, None]
    b = np.sqrt(w)
    g, *_ = np.linalg.lstsq(Aw, b, rcond=None)
    return g, lo, hi


def _szego_corr(sig2, ell, var):
    """Host-side scalar: sum_{k>=1} k c_k^2 for the symbol of A (pure
    function of the hyperparameters, like the gamma schedule)."""
    N = 65536
    th = 2.0 * np.pi * np.arange(N) / N
    s = np.zeros(N)
    for m in (-2, -1, 0, 1, 2):
        s += np.exp(-((ell * (th - 2.0 * np.pi * m)) ** 2) / 2.0)
    f = sig2 + var * ell * math.sqrt(2.0 * math.pi) * s
    ck = np.fft.rfft(np.log(f)).real / N
    k = np.arange(1, 4097)
    return float(np.sum(k * ck[1:4097] ** 2))


def _gtab_array(sig2, ell, var):
    """[P, GW] table: col j = gamma_j (j < N_DEG); col GW-1 = 1.0
    (the all-ones stationary column for the partition-sum matmul)."""
    gam, _, _ = _ls_poly_U(sig2, ell, var, N_DEG)
    row = np.zeros(GW, dtype=np.float32)
    row[:N_DEG] = gam
    row[GW - 1] = 1.0
    return np.tile(row[None, :], (P, 1))


DEFAULT_VARIANT = "U"


def make_in_map(sig2, ell, var, y, variant=DEFAULT_VARIANT):
    im = {"y": np.ascontiguousarray(np.asarray(y, dtype=np.float32))}
    if not variant.startswith("U"):
        im["gtab"] = _gtab_array(sig2, ell, var)
    return im


def _build(sig2, ell, var, n_copies=1, debug=False, variant="A"):
    """Emit the full program into a fresh Bacc instance and return it."""
    import concourse.mybir as mybir
    import concourse.tile as tile
    from concourse import bacc

    f32 = mybir.dt.float32

    nc = bacc.Bacc("TRN2", target_bir_lowering=False, debug=False)
    y_dram = nc.dram_tensor("y", [T], f32, kind="ExternalInput")
    gtab_dram = (
        None
        if variant.startswith("U")
        else nc.dram_tensor("gtab", [P, GW], f32, kind="ExternalInput")
    )
    out_dram = nc.dram_tensor("out", [1, n_copies], f32, kind="ExternalOutput")
    dbg = (
        nc.dram_tensor("dbg", [P, NBLK + 1], f32, kind="ExternalOutput")
        if debug
        else None
    )

    with tile.TileContext(nc) as tc:
        with (
            tc.tile_pool(name="const", bufs=1) as cpool,
            tc.tile_pool(name="work", bufs=1) as wpool,
            tc.tile_pool(name="ps", bufs=1, space="PSUM") as ppool,
        ):
            emit = (_emit_one_u if variant.startswith("U")
                    else _emit_one if variant in ("A", "D") else _emit_one_b)
            for ci in range(n_copies):
                emit(
                    nc, tc, cpool, wpool, ppool, mybir,
                    y_dram, gtab_dram, out_dram, dbg if ci == 0 else None,
                    sig2, ell, var, ci, variant,
                )

    nc.compile()
    return nc


def _emit_one(
    nc, tc, cpool, wpool, ppool, mybir,
    y_dram, gtab_dram, out_dram, dbg,
    sig2, ell, var, ci, variant="A",
):
    """Variant A: hardware For_i loop, one recurrence step per trip.
    Trip i computes q_{i+1} = (2As) q_i - q_{i-1} into slot i+2 and
    accumulates x += gamma_i q_i (slot i+1), so no xs-init op is needed.
    Szego grid recycled from the squared NS2 iota; single merged reduce."""
    from concourse.bass import ds

    f32 = mybir.dt.float32
    AF = mybir.ActivationFunctionType
    OP = mybir.AluOpType

    gam, lam_lo, lam_hi = _ls_poly_U(sig2, ell, var, N_DEG)
    sc2 = 4.0 / (lam_hi - lam_lo)
    sh2 = -2.0 * (lam_hi + lam_lo) / (lam_hi - lam_lo)
    corr = _szego_corr(sig2, ell, var) + _szego_missing(sig2, ell, var)

    gtab = cpool.tile([P, GW], f32, tag=f"gtab{ci}")
    nc.sync.dma_start(gtab[:], gtab_dram[:])

    # slots: 0 = q_{-1} (zero), 1 = q_0 = y, ..., NSLOT-2 = q_{N_DEG}
    # (extra, unused), NSLOT-1 = x accumulator
    big = wpool.tile([P, NSLOT * SW], f32, tag=f"big{ci}")
    nc.vector.memset(big[:], 0.0)
    yw = big[:, SW + 1 : SW + 1 + NBLK]
    nc.sync.dma_start(yw, y_dram.rearrange("(b r) -> r b", b=NBLK))
    XO = (NSLOT - 1) * SW
    xt = big[:, XO : XO + SW]
    xw = big[:, XO + 1 : XO + 1 + NBLK]

    NS2 = cpool.tile([P, 3, P], f32, tag=f"NS2{ci}")
    nc.gpsimd.iota(
        NS2[:], pattern=[[P, 3], [-1, P]], base=-P, channel_multiplier=1,
        allow_small_or_imprecise_dtypes=True,
    )
    nc.scalar.activation(NS2[:], NS2[:], AF.Square)
    t0 = wpool.tile([P, NBLK + 3], f32, tag=f"t0{ci}")
    gl = t0[:, NBLK : NBLK + 3]
    th_sc = (2.0 * math.pi / N_GRID) ** 2 * ell * ell / 2.0
    nc.scalar.activation(gl, NS2[:, :, 0], AF.Exp, scale=float(-th_sc))
    nc.scalar.activation(
        gl, gl, AF.Ln,
        scale=float(var * ell * math.sqrt(2.0 * math.pi)),
        bias=_bias_arg(nc, cpool, mybir, sig2, f"sgb{ci}"),
    )
    nc.vector.tensor_scalar(gl, gl, float(-0.5 * T / N_GRID), None, op0=OP.mult)
    nc.scalar.activation(
        NS2[:], NS2[:], AF.Exp, scale=float(-1.0 / (2.0 * ell * ell))
    )
    nc.vector.tensor_scalar(
        NS2[:], NS2[:], float(sc2 * var), None, op0=OP.mult
    )
    nc.gpsimd.affine_select(
        out=NS2[:, 1, :], in_=NS2[:, 1, :],
        compare_op=mybir.AluOpType.not_equal,
        fill=float(sc2 * (var + sig2) + sh2),
        base=0, pattern=[[-1, P]], channel_multiplier=1,
    )

    W_ps = ppool.tile([P, NBLK], f32, tag="W_ps")

    with tc.For_i(0, N_DEG, 1, staggered_reset=(variant == "D")) as i:
        for m in range(3):
            nc.tensor.matmul(
                W_ps[:],
                NS2[:, m, :],
                big[:, ds(i * SW + SW + m, NBLK)],
                start=(m == 0),
                stop=(m == 2),
                skip_group_check=True,
            )
        nc.vector.scalar_tensor_tensor(
            big[:, ds(i * SW + 2 * SW + 1, NBLK)],
            in0=W_ps[:], scalar=1.0, in1=big[:, ds(i * SW + 1, NBLK)],
            op0=OP.mult, op1=OP.subtract,
        )
        nc.vector.scalar_tensor_tensor(
            xw, in0=big[:, ds(i * SW + SW + 1, NBLK)],
            scalar=gtab[:, ds(i, 1)], in1=xw,
            op0=OP.mult, op1=OP.add,
        )

    # quad tail (same as U): t0[:, 0:64] = -0.5 * x * (2y - A x)
    for m in range(3):
        nc.tensor.matmul(
            W_ps[:], NS2[:, m, :], big[:, XO + m : XO + m + NBLK],
            start=(m == 0), stop=(m == 2), skip_group_check=True,
        )
    tq = t0[:, 0:NBLK]
    nc.vector.scalar_tensor_tensor(
        tq, in0=xw, scalar=float(sh2), in1=W_ps[:],
        op0=OP.mult, op1=OP.subtract,
    )
    nc.vector.scalar_tensor_tensor(
        tq, in0=tq, scalar=float(-0.5 / sc2), in1=yw,
        op0=OP.mult, op1=OP.subtract,
    )
    nc.vector.tensor_tensor(tq, tq, xw, op=OP.mult)

    red = wpool.tile([P, 1], f32, tag=f"red{ci}")
    nc.vector.tensor_reduce(red[:], t0[:], axis=mybir.AxisListType.X, op=OP.add)
    out_ps = ppool.tile([1, 1], f32, tag="out_ps")
    nc.tensor.matmul(
        out_ps[:], gtab[:, GW - 1 : GW], red[:], start=True, stop=True,
        skip_group_check=True,
    )
    fin = wpool.tile([1, 1], f32, tag=f"fin{ci}")
    nc.vector.tensor_scalar(
        fin[:], out_ps[:], 1.0, float(-0.5 * corr), op0=OP.mult, op1=OP.add
    )
    nc.sync.dma_start(out_dram[:, ci : ci + 1], fin[:])


def get_program(sig2, ell, var, n_copies=1, debug=False, variant=DEFAULT_VARIANT):
    key = (float(sig2), float(ell), float(var), int(n_copies), bool(debug), variant)
    if key not in _prog_cache:
        _prog_cache[key] = _build(
            *key[:3], n_copies=key[3], debug=key[4], variant=key[5]
        )
    return _prog_cache[key]


def kernel(y, sigma_sq, lengthscale, variance):
    from concourse import bass_utils

    y = np.ascontiguousarray(np.asarray(y, dtype=np.float32))
    sig2 = float(np.asarray(sigma_sq).reshape(-1)[0])
    ell = float(np.asarray(lengthscale))
    var = float(np.asarray(variance))
    assert y.shape == (T,)

    nc = get_program(sig2, ell, var)
    in_map = make_in_map(sig2, ell, var, y)

    res = bass_utils.run_bass_kernel_spmd(
        nc, [dict(in_map) for _ in range(8)], core_ids=list(range(8))
    )
    out = res.results[0]["out"]
    return np.asarray(out, dtype=np.float32).reshape(1, 1)


if __name__ == "__main__":
    rng = np.random.default_rng(0)
    y = rng.standard_normal(T).astype(np.float32)
    o = kernel(y, np.ones(1, np.float32), np.float32(32.0), np.float32(1.0))
    print("kernel out:", o)


def _emit_one_b(
    nc, tc, cpool, wpool, ppool, mybir,
    y_dram, gtab_dram, out_dram, dbg,
    sig2, ell, var, ci, variant="B",
):
    """Variant B: double-step For_i loop (NTRIP/2 trips), static matmul
    operands (vb/vc slots), ds() only on the gamma scalar APs."""
    from concourse.bass import ds

    f32 = mybir.dt.float32
    AF = mybir.ActivationFunctionType
    OP = mybir.AluOpType

    gam, lam_lo, lam_hi = _ls_poly_U(sig2, ell, var, N_DEG)
    sc2 = 4.0 / (lam_hi - lam_lo)
    sh2 = -2.0 * (lam_hi + lam_lo) / (lam_hi - lam_lo)
    corr = _szego_corr(sig2, ell, var)

    gtab = cpool.tile([P, GW], f32, tag=f"gtab{ci}")
    nc.sync.dma_start(gtab[:], gtab_dram[:])
    # B-layout gamma table: col j = gamma_{2j+1}, col 8+j = gamma_{2j+2}
    # (host array handled by _gtab_array_b); ones col GW-1.
    godd = gtab[:, 0:12]
    gevn = gtab[:, 12:24]

    big = wpool.tile([P, 4 * SW], f32, tag=f"big{ci}")
    nc.vector.memset(big[:], 0.0)
    vy = big[:, 0 * SW : 0 * SW + SW]
    vb = big[:, 1 * SW : 1 * SW + SW]
    vc = big[:, 2 * SW : 2 * SW + SW]
    xt = big[:, 3 * SW : 3 * SW + SW]
    yw = vy[:, 1 : 1 + NBLK]
    nc.sync.dma_start(yw, y_dram.rearrange("(b r) -> r b", b=NBLK))
    nc.sync.dma_start(vb[:, 1 : 1 + NBLK], y_dram.rearrange("(b r) -> r b", b=NBLK))
    xw = xt[:, 1 : 1 + NBLK]
    nc.vector.tensor_scalar(xw, yw, float(gam[0]), None, op0=OP.mult)

    NS2 = cpool.tile([P, 3, P], f32, tag=f"NS2{ci}")
    nc.gpsimd.iota(
        NS2[:], pattern=[[P, 3], [-1, P]], base=-P, channel_multiplier=1,
        allow_small_or_imprecise_dtypes=True,
    )
    nc.scalar.activation(NS2[:], NS2[:], AF.Square)
    nc.scalar.activation(
        NS2[:], NS2[:], AF.Exp, scale=float(-1.0 / (2.0 * ell * ell))
    )
    nc.vector.tensor_scalar(
        NS2[:], NS2[:], float(sc2 * var), None, op0=OP.mult
    )
    nc.gpsimd.affine_select(
        out=NS2[:, 1, :], in_=NS2[:, 1, :],
        compare_op=mybir.AluOpType.not_equal,
        fill=float(sc2 * (var + sig2) + sh2),
        base=0, pattern=[[-1, P]], channel_multiplier=1,
    )

    pair = wpool.tile([P, 2], f32, tag=f"pair{ci}")
    gl = wpool.tile([P, NGC], f32, tag=f"gl{ci}")
    nc.gpsimd.iota(
        gl[:], pattern=[[P, NGC]], base=-N_GRID // 2, channel_multiplier=1,
        allow_small_or_imprecise_dtypes=True,
    )
    nc.scalar.activation(
        gl[:], gl[:], AF.Square, scale=float(2.0 * math.pi / N_GRID)
    )
    nc.scalar.activation(
        gl[:], gl[:], AF.Exp, scale=float(-(ell * ell) / 2.0)
    )
    nc.scalar.activation(
        gl[:], gl[:], AF.Ln,
        scale=float(var * ell * math.sqrt(2.0 * math.pi)), bias=float(sig2),
    )
    nc.vector.tensor_reduce(
        pair[:, 1:2], gl[:], axis=mybir.AxisListType.X, op=OP.add
    )

    W_ps = ppool.tile([P, NBLK], f32, tag="W_ps")

    def matvec(src):
        for m in range(3):
            nc.tensor.matmul(
                W_ps[:], NS2[:, m, :], src[:, m : m + NBLK],
                start=(m == 0), stop=(m == 2), skip_group_check=True,
            )

    assert NTRIP % 2 == 0
    with tc.For_i(0, NTRIP // 2, 1) as i:
        matvec(vb)
        nc.vector.scalar_tensor_tensor(
            vc[:, 1 : 1 + NBLK], in0=W_ps[:], scalar=1.0,
            in1=vc[:, 1 : 1 + NBLK], op0=OP.mult, op1=OP.subtract,
        )
        nc.vector.scalar_tensor_tensor(
            xw, in0=vc[:, 1 : 1 + NBLK], scalar=godd[:, ds(i, 1)], in1=xw,
            op0=OP.mult, op1=OP.add,
        )
        matvec(vc)
        nc.vector.scalar_tensor_tensor(
            vb[:, 1 : 1 + NBLK], in0=W_ps[:], scalar=1.0,
            in1=vb[:, 1 : 1 + NBLK], op0=OP.mult, op1=OP.subtract,
        )
        nc.vector.scalar_tensor_tensor(
            xw, in0=vb[:, 1 : 1 + NBLK], scalar=gevn[:, ds(i, 1)], in1=xw,
            op0=OP.mult, op1=OP.add,
        )

    matvec(xt)
    t0 = wpool.tile([P, NBLK], f32, tag=f"t0{ci}")
    nc.vector.scalar_tensor_tensor(
        t0[:], in0=xw, scalar=float(sh2), in1=W_ps[:],
        op0=OP.mult, op1=OP.subtract,
    )
    nc.vector.scalar_tensor_tensor(
        t0[:], in0=t0[:], scalar=float(1.0 / sc2), in1=yw,
        op0=OP.mult, op1=OP.add,
    )
    nc.vector.tensor_tensor(t0[:], t0[:], yw, op=OP.add)
    nc.vector.tensor_tensor(t0[:], t0[:], xw, op=OP.mult)
    nc.vector.tensor_reduce(
        pair[:, 0:1], t0[:], axis=mybir.AxisListType.X, op=OP.add
    )

    out_ps = ppool.tile([1, 2], f32, tag="out_ps")
    nc.tensor.matmul(
        out_ps[:], gtab[:, GW - 1 : GW], pair[:], start=True, stop=True,
        skip_group_check=True,
    )
    fin = wpool.tile([1, 2], f32, tag=f"fin{ci}")
    nc.vector.tensor_scalar(
        fin[:, 1:2], out_ps[:, 1:2],
        float(-0.5 * T / N_GRID), float(-0.5 * corr),
        op0=OP.mult, op1=OP.add,
    )
    nc.vector.scalar_tensor_tensor(
        fin[:, 0:1], in0=out_ps[:, 0:1], scalar=-0.5, in1=fin[:, 1:2],
        op0=OP.mult, op1=OP.add,
    )
    nc.sync.dma_start(out_dram[:, ci : ci + 1], fin[:, 0:1])


def _gtab_array_b(sig2, ell, var):
    gam, _, _ = _ls_poly_U(sig2, ell, var, N_DEG)
    row = np.zeros(GW, dtype=np.float32)
    for j in range(12):
        if 2 * j + 1 < N_DEG:
            row[j] = gam[2 * j + 1]
        if 2 * j + 2 < N_DEG:
            row[12 + j] = gam[2 * j + 2]
    row[GW - 1] = 1.0
    return np.tile(row[None, :], (P, 1))


def _log_symbol(th, sig2, ell, var):
    s = np.zeros(np.shape(th))
    for m in (-3, -2, -1, 0, 1, 2, 3):
        s += np.exp(-((ell * (th - 2.0 * np.pi * m)) ** 2) / 2.0)
    return np.log(sig2 + var * ell * math.sqrt(2.0 * math.pi) * s)


def _szego_missing(sig2, ell, var, noff):
    """Host-side scalar: (T/N_GRID) * (full-circle sum of log f minus the
    sum over the on-device NS2-derived grid j = p + 128*(m - noff//2)).
    Pure hyperparameter function, exact in f64."""
    j_full = np.arange(-N_GRID // 2, N_GRID // 2)
    j_dev = np.concatenate([
        np.arange(P) + P * (m - noff // 2) for m in range(noff)
    ])
    full = np.sum(_log_symbol(2.0 * np.pi * j_full / N_GRID, sig2, ell, var))
    dev = np.sum(_log_symbol(2.0 * np.pi * j_dev / N_GRID, sig2, ell, var))
    return float(T / N_GRID * (full - dev))


def _auto_plan(sig2, ell, var):
    """Pick polynomial degree (from the spectrum condition number, so the
    second-order functional keeps ~30x margin under the 2e-2 gate) and
    band block count (3 unless the |i-j| in [129,255] tail that the
    3-block operator drops in block corners is non-negligible)."""
    lam_hi = sig2 + var * ell * math.sqrt(2.0 * math.pi)
    kappa = max(lam_hi / sig2, 1.0 + 1e-6)
    rho = (math.sqrt(kappa) - 1.0) / (math.sqrt(kappa) + 1.0)
    n_deg = max(6, min(48, math.ceil(math.log(0.14) / math.log(max(rho, 1e-6)))))
    # the dropped band tail enters the quad error quadratically (the
    # second-order functional), so 3 blocks are fine until ~2e-3
    tail = var * math.exp(-((P + 1) ** 2) / (2.0 * ell * ell)) / sig2
    noff = 3 if tail < 2e-3 else 5
    return n_deg, noff



def _bias_arg(nc, cpool, mybir, value, tag):
    """Activation bias: floats 0/1 pass through (registered const APs);
    anything else gets a dedicated [P,1] memset tile."""
    if float(value) in (0.0, 1.0):
        return float(value)
    t = cpool.tile([P, 1], mybir.dt.float32, tag=tag)
    nc.vector.memset(t[:], float(value))
    return t[:]

def _emit_one_u(
    nc, tc, cpool, wpool, ppool, mybir,
    y_dram, gtab_dram, out_dram, dbg,
    sig2, ell, var, ci, variant="U13",
):
    """Variant U: fully unrolled recurrence (no For_i), gamma as immediate
    floats, static buffer rotation (y loaded once; the second step
    subtracts q_0 directly from the preserved y tile), Szego grid recycled
    from the squared NS2 iota, single merged reduce."""
    f32 = mybir.dt.float32
    AF = mybir.ActivationFunctionType
    OP = mybir.AluOpType

    auto_deg, noff = _auto_plan(sig2, ell, var)
    n_deg = int(variant[1:]) if len(variant) > 1 else auto_deg
    hf = noff // 2
    gam, lam_lo, lam_hi = _ls_poly_U(sig2, ell, var, n_deg)
    sc2 = 4.0 / (lam_hi - lam_lo)
    sh2 = -2.0 * (lam_hi + lam_lo) / (lam_hi - lam_lo)
    corr = _szego_corr(sig2, ell, var) + _szego_missing(sig2, ell, var, noff)
    # gamma0-normalized schedule: the device accumulates x~ = x/gamma0
    # (so step 1 can fuse the x init: x~ = y + g1 q_1); gamma0 reappears
    # in the ub scalar and in the ones column of the partition reduce.
    g0 = float(gam[0])
    gn = [float(g / gam[0]) for g in gam]

    # big: vy | vb | vc | xs slots (window cols hf..hf+63, hf pads each side)
    usw = NBLK + 2 * hf
    big = wpool.tile([P, 4 * usw], f32, tag=f"big{ci}")
    nc.vector.memset(big[:], 0.0)
    vy = big[:, 0 * usw : 1 * usw]
    vb = big[:, 1 * usw : 2 * usw]
    vc = big[:, 2 * usw : 3 * usw]
    xt = big[:, 3 * usw : 4 * usw]
    yw = vy[:, hf : hf + NBLK]
    nc.sync.dma_start(yw, y_dram.rearrange("(b r) -> r b", b=NBLK))
    xw = xt[:, hf : hf + NBLK]

    # NS2 build; the squared iota doubles as the Szego grid source
    NS2 = cpool.tile([P, noff, P], f32, tag=f"NS2{ci}")
    nc.gpsimd.iota(
        NS2[:], pattern=[[P, noff], [-1, P]], base=-hf * P,
        channel_multiplier=1, allow_small_or_imprecise_dtypes=True,
    )
    nc.scalar.activation(NS2[:], NS2[:], AF.Square)
    # Szego: grid j = p + 128(m-1) at NS2[:, :, 0] (values j^2 after
    # Square); log f = ln(v l sqrt(2pi) exp(-(2pi j/N)^2 l^2/2) + sig2),
    # prescaled by T/N and parked in t0 cols 64..66 for the merged reduce.
    t0 = wpool.tile([P, NBLK + noff], f32, tag=f"t0{ci}")
    gl = t0[:, NBLK : NBLK + noff]
    th_sc = (2.0 * math.pi / N_GRID) ** 2 * ell * ell / 2.0
    nc.scalar.activation(gl, NS2[:, :, 0], AF.Exp, scale=float(-th_sc))
    nc.scalar.activation(
        gl, gl, AF.Ln,
        scale=float(var * ell * math.sqrt(2.0 * math.pi)),
        bias=_bias_arg(nc, cpool, mybir, sig2, f"sgb{ci}"),
    )
    nc.vector.tensor_scalar(
        gl, gl, float(-0.5 * T / N_GRID / g0), None, op0=OP.mult
    )
    # rest of NS2 build
    nc.scalar.activation(
        NS2[:], NS2[:], AF.Exp, scale=float(-1.0 / (2.0 * ell * ell))
    )
    nc.vector.tensor_scalar(
        NS2[:], NS2[:], float(sc2 * var), None, op0=OP.mult
    )
    nc.gpsimd.affine_select(
        out=NS2[:, hf, :], in_=NS2[:, hf, :],
        compare_op=mybir.AluOpType.not_equal,
        fill=float(sc2 * (var + sig2) + sh2),
        base=0, pattern=[[-1, P]], channel_multiplier=1,
    )

    W_ps = ppool.tile([P, NBLK], f32, tag="W_ps")

    def matvec(src):
        for m in range(noff):
            nc.tensor.matmul(
                W_ps[:], NS2[:, m, :], src[:, m : m + NBLK],
                start=(m == 0), stop=(m == noff - 1), skip_group_check=True,
            )

    # unrolled recurrence; q_0 lives only in vy (read-only):
    #   step 1: q_1 = (2As) vy - vc(0) -> vc
    #   step 2: q_2 = (2As) vc - vy    -> vb   (in1 = vy, out = vb)
    #   step m>2: standard (vc, vb) rotation
    for m in range(1, n_deg):
        if m == 1:
            srcv, subv, dstv = vy, vc, vc
        elif m == 2:
            srcv, subv, dstv = vc, vy, vb
        elif m % 2 == 1:
            srcv, subv, dstv = vb, vc, vc
        else:
            srcv, subv, dstv = vc, vb, vb
        matvec(srcv)
        nc.vector.scalar_tensor_tensor(
            dstv[:, hf : hf + NBLK], in0=W_ps[:], scalar=1.0,
            in1=subv[:, hf : hf + NBLK], op0=OP.mult, op1=OP.subtract,
        )
        # m == 1 fuses the x init: x~ = 1.0*y + gn[1] q_1
        nc.vector.scalar_tensor_tensor(
            xw, in0=dstv[:, hf : hf + NBLK], scalar=float(gn[m]),
            in1=(yw if m == 1 else xw), op0=OP.mult, op1=OP.add,
        )

    # quad: t0[:, 0:64] = -0.5 * x * (2y - A x)
    #   ua = sh2*x - (2As)x;  ub = (-0.5/sc2)*ua - y;  tq = ub * x
    matvec(xt)
    tq = t0[:, 0:NBLK]
    nc.vector.scalar_tensor_tensor(
        tq, in0=xw, scalar=float(sh2), in1=W_ps[:],
        op0=OP.mult, op1=OP.subtract,
    )
    nc.vector.scalar_tensor_tensor(
        tq, in0=tq, scalar=float(-0.5 * g0 / sc2), in1=yw,
        op0=OP.mult, op1=OP.subtract,
    )
    nc.vector.tensor_tensor(tq, tq, xw, op=OP.mult)

    # merged reduce; the ones column carries gamma0 so that
    # g0 * (tq~ | gl~) sums to -0.5*quad - (T/2N)*sum(log f)
    red = wpool.tile([P, 1], f32, tag=f"red{ci}")
    nc.vector.tensor_reduce(red[:], t0[:], axis=mybir.AxisListType.X, op=OP.add)
    ones = cpool.tile([P, 1], f32, tag=f"ones{ci}")
    nc.vector.memset(ones[:], float(g0))
    out_ps = ppool.tile([1, 1], f32, tag="out_ps")
    nc.tensor.matmul(
        out_ps[:], ones[:], red[:], start=True, stop=True,
        skip_group_check=True,
    )
    fin = wpool.tile([1, 1], f32, tag=f"fin{ci}")
    nc.vector.tensor_scalar(
        fin[:], out_ps[:], 1.0, float(-0.5 * corr), op0=OP.mult, op1=OP.add
    )
    nc.sync.dma_start(out_dram[:, ci : ci + 1], fin[:])


# revision 16
# speedup vs baseline: 1.2767x; 1.2767x over previous
"""GP marginal log-likelihood kernel for Trainium2 (Bass/Tile).

Computes -0.5 * y^T A^-1 y - 0.5 * logdet(A) for A = K + sigma^2 I where
K is the RBF covariance on the integer grid 0..T-1 (T=8192).

A is symmetric positive-definite *Toeplitz* and effectively *banded*
(entries vanish below f32 eps for |i-j| > 255 at lengthscale 32), and is
well conditioned: eig(A) in [sigma^2, sigma^2 + v*sum_d k(d)] (~[1, 81.2]).
This kernel exploits that structure instead of a dense 8192^3 factorization:

  * quad = y^T A^-1 y: x = p(A) y where p is a least-squares-optimal
    polynomial fitted on the host (from the hyperparameters alone) to the
    model spectrum of A (the symbol samples), expressed in the Chebyshev
    *second-kind* basis U_m so the device recurrence is the uniform
    three-term form with no special first step:
        q_0 = y,  q_{m+1} = (2As) q_m - q_{m-1}  (q_{-1} = 0),
        x = sum_m gamma_m q_m,
    where each (2As) q is a block-tridiagonal matvec: 3 tensor-engine
    matmuls with 128x128 stationary band blocks (the |i-j| in [129,255]
    tail that falls outside +-1 block reach is ~3e-4 and verified
    numerically to not matter at the required tolerance).  The recurrence
    runs in a *hardware For_i loop*, one step per trip: the q_m live in
    one big SBUF tile as 66-column padded slots addressed by the loop
    variable, and gamma_m comes from a loop-indexed scalar AP.  The
    second-order functional quad = x^T (2y - A x) (with A x recovered
    from the same 2As operator: A x = ((2As)x - sh2 x)/sc2) makes the
    final error quadratic in the solver error (~5e-6 relative, degree 18).
  * logdet via the strong Szego limit theorem:
        logdet A = T*c_0 + sum_{k>=1} k*c_k^2,   c_k = Fourier coeffs of
    log f, f = the symbol of A.  T*c_0 is computed on device as the
    periodic-trapezoid mean of log f over a symmetric 512-point grid in
    [-pi, pi) (spectrally accurate; single Gaussian image suffices there).
    The small correction sum_k k*c_k^2 (~1.8 here) depends only on the
    scalar hyperparameters and is folded into the final combine constant
    on the host, like the polynomial coefficient schedule.

Everything data(y)-dependent runs on the device.  The host computes only
the iteration coefficient schedule and scalar constants from the scalar
hyperparameters (sigma^2, lengthscale, variance).  All 8 cores run the
same program on replicated inputs (the answer is a single scalar; core 0's
result is gathered).
"""

import math

import numpy as np

T = 8192
P = 128  # partitions
NBLK = T // P  # 64 column blocks
SW = NBLK + 2  # 66: one pad column each side of a 64-block slot
N_GRID = 512  # Szego c0 quadrature grid (symmetric, full circle)
NGC = N_GRID // P  # 4 grid columns
N_DEG = 19  # number of polynomial coefficients (highest index 18)
NTRIP = N_DEG - 1  # hardware-loop trips, one recurrence step each
NSLOT = N_DEG + 3  # q_{-1}(=0), q_0..q_{N_DEG}, x accumulator
GW = 32  # gamma table width

_prog_cache = {}


def _ls_poly_U(sig2, ell, var, n_deg):
    """Host-side schedule: LS-optimal solve polynomial in the U basis.

    Fits p(lam) = sum_m gamma_m U_m(s(lam)) minimizing
    sum_j (1 - lam_j p(lam_j))^2 / lam_j over the model spectrum
    lam_j = f(2 pi j / T) (symbol samples, the asymptotic eigenvalue
    distribution of A).  Returns (gamma, lo, hi)."""
    th = np.linspace(0.0, np.pi, T // 2 + 1)
    lam = sig2 + var * ell * math.sqrt(2.0 * math.pi) * (
        np.exp(-((ell * th) ** 2) / 2.0)
        + np.exp(-((ell * (th - 2 * math.pi)) ** 2) / 2.0)
    )
    lo, hi = float(lam.min()), float(lam.max())
    xs = (2.0 * lam - (hi + lo)) / (hi - lo)
    V = np.zeros((lam.size, n_deg))
    V[:, 0] = 1.0
    if n_deg > 1:
        V[:, 1] = 2.0 * xs  # U_1 = 2x
    for m in range(2, n_deg):
        V[:, m] = 2.0 * xs * V[:, m - 1] - V[:, m - 2]
    w = 1.0 / lam
    Aw = V * (lam * np.sqrt(w))[:, None]
    b = np.sqrt(w)
    g, *_ = np.linalg.lstsq(Aw, b, rcond=None)
    return g, lo, hi


def _szego_corr(sig2, ell, var):
    """Host-side scalar: sum_{k>=1} k c_k^2 for the symbol of A (pure
    function of the hyperparameters, like the gamma schedule)."""
    N = 65536
    th = 2.0 * np.pi * np.arange(N) / N
    s = np.zeros(N)
    for m in (-2, -1, 0, 1, 2):
        s += np.exp(-((ell * (th - 2.0 * np.pi * m)) ** 2) / 2.0)
    f = sig2 + var * ell * math.sqrt(2.0 * math.pi) * s
    ck = np.fft.rfft(np.log(f)).real / N
    k = np.arange(1, 4097)
    return float(np.sum(k * ck[1:4097] ** 2))


def _gtab_array(sig2, ell, var):
    """[P, GW] table: col j = gamma_j (j < N_DEG); col GW-1 = 1.0
    (the all-ones stationary column for the partition-sum matmul)."""
    gam, _, _ = _ls_poly_U(sig2, ell, var, N_DEG)
    row = np.zeros(GW, dtype=np.float32)
    row[:N_DEG] = gam
    row[GW - 1] = 1.0
    return np.tile(row[None, :], (P, 1))


DEFAULT_VARIANT = "U"


def make_in_map(sig2, ell, var, y, variant=DEFAULT_VARIANT):
    im = {"y": np.ascontiguousarray(np.asarray(y, dtype=np.float32))}
    if not variant.startswith("U"):
        im["gtab"] = _gtab_array(sig2, ell, var)
    return im


def _build(sig2, ell, var, n_copies=1, debug=False, variant="A"):
    """Emit the full program into a fresh Bacc instance and return it."""
    import concourse.mybir as mybir
    import concourse.tile as tile
    from concourse import bacc

    f32 = mybir.dt.float32

    nc = bacc.Bacc("TRN2", target_bir_lowering=False, debug=False)
    y_dram = nc.dram_tensor("y", [T], f32, kind="ExternalInput")
    gtab_dram = (
        None
        if variant.startswith("U")
        else nc.dram_tensor("gtab", [P, GW], f32, kind="ExternalInput")
    )
    out_dram = nc.dram_tensor("out", [1, n_copies], f32, kind="ExternalOutput")
    dbg = (
        nc.dram_tensor("dbg", [P, NBLK + 1], f32, kind="ExternalOutput")
        if debug
        else None
    )

    with tile.TileContext(nc) as tc:
        with (
            tc.tile_pool(name="const", bufs=1) as cpool,
            tc.tile_pool(name="work", bufs=1) as wpool,
            tc.tile_pool(name="ps", bufs=1, space="PSUM") as ppool,
        ):
            emit = _emit_one_u if variant.startswith("U") else _emit_one
            for ci in range(n_copies):
                emit(
                    nc, tc, cpool, wpool, ppool, mybir,
                    y_dram, gtab_dram, out_dram, dbg if ci == 0 else None,
                    sig2, ell, var, ci, variant,
                )

    nc.compile()
    return nc


def _emit_one(
    nc, tc, cpool, wpool, ppool, mybir,
    y_dram, gtab_dram, out_dram, dbg,
    sig2, ell, var, ci, variant="A",
):
    """Variant A: hardware For_i loop, one recurrence step per trip.
    Trip i computes q_{i+1} = (2As) q_i - q_{i-1} into slot i+2 and
    accumulates x += gamma_i q_i (slot i+1), so no xs-init op is needed.
    Szego grid recycled from the squared NS2 iota; single merged reduce."""
    from concourse.bass import ds

    f32 = mybir.dt.float32
    AF = mybir.ActivationFunctionType
    OP = mybir.AluOpType

    gam, lam_lo, lam_hi = _ls_poly_U(sig2, ell, var, N_DEG)
    sc2 = 4.0 / (lam_hi - lam_lo)
    sh2 = -2.0 * (lam_hi + lam_lo) / (lam_hi - lam_lo)
    corr = _szego_corr(sig2, ell, var) + _szego_missing(sig2, ell, var)

    gtab = cpool.tile([P, GW], f32, tag=f"gtab{ci}")
    nc.sync.dma_start(gtab[:], gtab_dram[:])

    # slots: 0 = q_{-1} (zero), 1 = q_0 = y, ..., NSLOT-2 = q_{N_DEG}
    # (extra, unused), NSLOT-1 = x accumulator
    big = wpool.tile([P, NSLOT * SW], f32, tag=f"big{ci}")
    nc.vector.memset(big[:], 0.0)
    yw = big[:, SW + 1 : SW + 1 + NBLK]
    nc.sync.dma_start(yw, y_dram.rearrange("(b r) -> r b", b=NBLK))
    XO = (NSLOT - 1) * SW
    xt = big[:, XO : XO + SW]
    xw = big[:, XO + 1 : XO + 1 + NBLK]

    NS2 = cpool.tile([P, 3, P], f32, tag=f"NS2{ci}")
    nc.gpsimd.iota(
        NS2[:], pattern=[[P, 3], [-1, P]], base=-P, channel_multiplier=1,
        allow_small_or_imprecise_dtypes=True,
    )
    nc.scalar.activation(NS2[:], NS2[:], AF.Square)
    t0 = wpool.tile([P, NBLK + 3], f32, tag=f"t0{ci}")
    gl = t0[:, NBLK : NBLK + 3]
    th_sc = (2.0 * math.pi / N_GRID) ** 2 * ell * ell / 2.0
    nc.scalar.activation(gl, NS2[:, :, 0], AF.Exp, scale=float(-th_sc))
    nc.scalar.activation(
        gl, gl, AF.Ln,
        scale=float(var * ell * math.sqrt(2.0 * math.pi)),
        bias=_bias_arg(nc, cpool, mybir, sig2, f"sgb{ci}"),
    )
    nc.vector.tensor_scalar(gl, gl, float(-0.5 * T / N_GRID), None, op0=OP.mult)
    nc.scalar.activation(
        NS2[:], NS2[:], AF.Exp, scale=float(-1.0 / (2.0 * ell * ell))
    )
    nc.vector.tensor_scalar(
        NS2[:], NS2[:], float(sc2 * var), None, op0=OP.mult
    )
    nc.gpsimd.affine_select(
        out=NS2[:, 1, :], in_=NS2[:, 1, :],
        compare_op=mybir.AluOpType.not_equal,
        fill=float(sc2 * (var + sig2) + sh2),
        base=0, pattern=[[-1, P]], channel_multiplier=1,
    )

    W_ps = ppool.tile([P, NBLK], f32, tag="W_ps")

    with tc.For_i(0, N_DEG, 1, staggered_reset=(variant == "D")) as i:
        for m in range(3):
            nc.tensor.matmul(
                W_ps[:],
                NS2[:, m, :],
                big[:, ds(i * SW + SW + m, NBLK)],
                start=(m == 0),
                stop=(m == 2),
                skip_group_check=True,
            )
        nc.vector.scalar_tensor_tensor(
            big[:, ds(i * SW + 2 * SW + 1, NBLK)],
            in0=W_ps[:], scalar=1.0, in1=big[:, ds(i * SW + 1, NBLK)],
            op0=OP.mult, op1=OP.subtract,
        )
        nc.vector.scalar_tensor_tensor(
            xw, in0=big[:, ds(i * SW + SW + 1, NBLK)],
            scalar=gtab[:, ds(i, 1)], in1=xw,
            op0=OP.mult, op1=OP.add,
        )

    # quad tail (same as U): t0[:, 0:64] = -0.5 * x * (2y - A x)
    for m in range(3):
        nc.tensor.matmul(
            W_ps[:], NS2[:, m, :], big[:, XO + m : XO + m + NBLK],
            start=(m == 0), stop=(m == 2), skip_group_check=True,
        )
    tq = t0[:, 0:NBLK]
    nc.vector.scalar_tensor_tensor(
        tq, in0=xw, scalar=float(sh2), in1=W_ps[:],
        op0=OP.mult, op1=OP.subtract,
    )
    nc.vector.scalar_tensor_tensor(
        tq, in0=tq, scalar=float(-0.5 / sc2), in1=yw,
        op0=OP.mult, op1=OP.subtract,
    )
    nc.vector.tensor_tensor(tq, tq, xw, op=OP.mult)

    red = wpool.tile([P, 1], f32, tag=f"red{ci}")
    nc.vector.tensor_reduce(red[:], t0[:], axis=mybir.AxisListType.X, op=OP.add)
    out_ps = ppool.tile([1, 1], f32, tag="out_ps")
    nc.tensor.matmul(
        out_ps[:], gtab[:, GW - 1 : GW], red[:], start=True, stop=True,
        skip_group_check=True,
    )
    fin = wpool.tile([1, 1], f32, tag=f"fin{ci}")
    nc.vector.tensor_scalar(
        fin[:], out_ps[:], 1.0, float(-0.5 * corr), op0=OP.mult, op1=OP.add
    )
    nc.sync.dma_start(out_dram[:, ci : ci + 1], fin[:])


def get_program(sig2, ell, var, n_copies=1, debug=False, variant=DEFAULT_VARIANT):
    key = (float(sig2), float(ell), float(var), int(n_copies), bool(debug), variant)
    if key not in _prog_cache:
        _prog_cache[key] = _build(
            *key[:3], n_copies=key[3], debug=key[4], variant=key[5]
        )
    return _prog_cache[key]


def kernel(y, sigma_sq, lengthscale, variance):
    from concourse import bass_utils

    y = np.ascontiguousarray(np.asarray(y, dtype=np.float32))
    sig2 = float(np.asarray(sigma_sq).reshape(-1)[0])
    ell = float(np.asarray(lengthscale))
    var = float(np.asarray(variance))
    assert y.shape == (T,)

    nc = get_program(sig2, ell, var)
    in_map = make_in_map(sig2, ell, var, y)

    res = bass_utils.run_bass_kernel_spmd(
        nc, [dict(in_map) for _ in range(8)], core_ids=list(range(8))
    )
    out = res.results[0]["out"]
    return np.asarray(out, dtype=np.float32).reshape(1, 1)


if __name__ == "__main__":
    rng = np.random.default_rng(0)
    y = rng.standard_normal(T).astype(np.float32)
    o = kernel(y, np.ones(1, np.float32), np.float32(32.0), np.float32(1.0))
    print("kernel out:", o)



def _log_symbol(th, sig2, ell, var):
    s = np.zeros(np.shape(th))
    for m in (-3, -2, -1, 0, 1, 2, 3):
        s += np.exp(-((ell * (th - 2.0 * np.pi * m)) ** 2) / 2.0)
    return np.log(sig2 + var * ell * math.sqrt(2.0 * math.pi) * s)


def _szego_missing(sig2, ell, var, noff=3):
    """Host-side scalar: (T/N_GRID) * (full-circle sum of log f minus the
    sum over the on-device NS2-derived grid j = p + 128*(m - noff//2)).
    Pure hyperparameter function, exact in f64."""
    j_full = np.arange(-N_GRID // 2, N_GRID // 2)
    j_dev = np.concatenate([
        np.arange(P) + P * (m - noff // 2) for m in range(noff)
    ])
    full = np.sum(_log_symbol(2.0 * np.pi * j_full / N_GRID, sig2, ell, var))
    dev = np.sum(_log_symbol(2.0 * np.pi * j_dev / N_GRID, sig2, ell, var))
    return float(T / N_GRID * (full - dev))


def _auto_plan(sig2, ell, var):
    """Pick polynomial degree (from the spectrum condition number, so the
    second-order functional keeps ~30x margin under the 2e-2 gate) and
    band block count (3 unless the |i-j| in [129,255] tail that the
    3-block operator drops in block corners is non-negligible)."""
    lam_hi = sig2 + var * ell * math.sqrt(2.0 * math.pi)
    kappa = max(lam_hi / sig2, 1.0 + 1e-6)
    rho = (math.sqrt(kappa) - 1.0) / (math.sqrt(kappa) + 1.0)
    n_deg = max(6, min(48, math.ceil(math.log(0.14) / math.log(max(rho, 1e-6)))))
    # the dropped band tail enters the quad error quadratically (the
    # second-order functional), so 3 blocks are fine until ~2e-3
    tail = var * math.exp(-((P + 1) ** 2) / (2.0 * ell * ell)) / sig2
    noff = 3 if tail < 2e-3 else 5
    return n_deg, noff


def _bias_arg(nc, cpool, mybir, value, tag):
    """Activation bias: floats 0/1 pass through (registered const APs);
    anything else gets a dedicated [P,1] memset tile."""
    if float(value) in (0.0, 1.0):
        return float(value)
    t = cpool.tile([P, 1], mybir.dt.float32, tag=tag)
    nc.vector.memset(t[:], float(value))
    return t[:]

def _emit_one_u(
    nc, tc, cpool, wpool, ppool, mybir,
    y_dram, gtab_dram, out_dram, dbg,
    sig2, ell, var, ci, variant="U13",
):
    """Variant U: fully unrolled recurrence (no For_i), gamma as immediate
    floats, static buffer rotation (y loaded once; the second step
    subtracts q_0 directly from the preserved y tile), Szego grid recycled
    from the squared NS2 iota, single merged reduce."""
    f32 = mybir.dt.float32
    AF = mybir.ActivationFunctionType
    OP = mybir.AluOpType

    auto_deg, noff = _auto_plan(sig2, ell, var)
    n_deg = int(variant[1:]) if len(variant) > 1 else auto_deg
    hf = noff // 2
    gam, lam_lo, lam_hi = _ls_poly_U(sig2, ell, var, n_deg)
    sc2 = 4.0 / (lam_hi - lam_lo)
    sh2 = -2.0 * (lam_hi + lam_lo) / (lam_hi - lam_lo)
    corr = _szego_corr(sig2, ell, var) + _szego_missing(sig2, ell, var, noff)
    # gamma0-normalized schedule: the device accumulates x~ = x/gamma0
    # (so step 1 can fuse the x init: x~ = y + g1 q_1); gamma0 reappears
    # in the ub scalar and in the ones column of the partition reduce.
    g0 = float(gam[0])
    gn = [float(g / gam[0]) for g in gam]

    # big: vy | vb | vc | xs slots (window cols hf..hf+63, hf pads each side)
    usw = NBLK + 2 * hf
    big = wpool.tile([P, 4 * usw], f32, tag=f"big{ci}")
    nc.vector.memset(big[:], 0.0)
    vy = big[:, 0 * usw : 1 * usw]
    vb = big[:, 1 * usw : 2 * usw]
    vc = big[:, 2 * usw : 3 * usw]
    xt = big[:, 3 * usw : 4 * usw]
    yw = vy[:, hf : hf + NBLK]
    nc.sync.dma_start(yw, y_dram.rearrange("(b r) -> r b", b=NBLK))
    xw = xt[:, hf : hf + NBLK]

    # NS2 build; the squared iota doubles as the Szego grid source
    NS2 = cpool.tile([P, noff, P], f32, tag=f"NS2{ci}")
    nc.gpsimd.iota(
        NS2[:], pattern=[[P, noff], [-1, P]], base=-hf * P,
        channel_multiplier=1, allow_small_or_imprecise_dtypes=True,
    )
    nc.scalar.activation(NS2[:], NS2[:], AF.Square)
    # Szego: grid j = p + 128(m-1) at NS2[:, :, 0] (values j^2 after
    # Square); log f = ln(v l sqrt(2pi) exp(-(2pi j/N)^2 l^2/2) + sig2),
    # prescaled by T/N and parked in t0 cols 64..66 for the merged reduce.
    t0 = wpool.tile([P, NBLK + noff], f32, tag=f"t0{ci}")
    gl = t0[:, NBLK : NBLK + noff]
    th_sc = (2.0 * math.pi / N_GRID) ** 2 * ell * ell / 2.0
    nc.scalar.activation(gl, NS2[:, :, 0], AF.Exp, scale=float(-th_sc))
    nc.scalar.activation(
        gl, gl, AF.Ln,
        scale=float(var * ell * math.sqrt(2.0 * math.pi)),
        bias=_bias_arg(nc, cpool, mybir, sig2, f"sgb{ci}"),
    )
    nc.vector.tensor_scalar(
        gl, gl, float(-0.5 * T / N_GRID / g0), None, op0=OP.mult
    )
    # rest of NS2 build
    nc.scalar.activation(
        NS2[:], NS2[:], AF.Exp, scale=float(-1.0 / (2.0 * ell * ell))
    )
    nc.vector.tensor_scalar(
        NS2[:], NS2[:], float(sc2 * var), None, op0=OP.mult
    )
    nc.gpsimd.affine_select(
        out=NS2[:, hf, :], in_=NS2[:, hf, :],
        compare_op=mybir.AluOpType.not_equal,
        fill=float(sc2 * (var + sig2) + sh2),
        base=0, pattern=[[-1, P]], channel_multiplier=1,
    )

    W_ps = ppool.tile([P, NBLK], f32, tag="W_ps")

    def matvec(src):
        for m in range(noff):
            nc.tensor.matmul(
                W_ps[:], NS2[:, m, :], src[:, m : m + NBLK],
                start=(m == 0), stop=(m == noff - 1), skip_group_check=True,
            )

    # unrolled recurrence; q_0 lives only in vy (read-only):
    #   step 1: q_1 = (2As) vy - vc(0) -> vc
    #   step 2: q_2 = (2As) vc - vy    -> vb   (in1 = vy, out = vb)
    #   step m>2: standard (vc, vb) rotation
    for m in range(1, n_deg):
        if m == 1:
            srcv, subv, dstv = vy, vc, vc
        elif m == 2:
            srcv, subv, dstv = vc, vy, vb
        elif m % 2 == 1:
            srcv, subv, dstv = vb, vc, vc
        else:
            srcv, subv, dstv = vc, vb, vb
        matvec(srcv)
        nc.vector.scalar_tensor_tensor(
            dstv[:, hf : hf + NBLK], in0=W_ps[:], scalar=1.0,
            in1=subv[:, hf : hf + NBLK], op0=OP.mult, op1=OP.subtract,
        )
        # m == 1 fuses the x init: x~ = 1.0*y + gn[1] q_1
        nc.vector.scalar_tensor_tensor(
            xw, in0=dstv[:, hf : hf + NBLK], scalar=float(gn[m]),
            in1=(yw if m == 1 else xw), op0=OP.mult, op1=OP.add,
        )

    # quad: t0[:, 0:64] = -0.5 * x * (2y - A x)
    #   ua = sh2*x - (2As)x;  ub = (-0.5/sc2)*ua - y;  tq = ub * x
    matvec(xt)
    tq = t0[:, 0:NBLK]
    nc.vector.scalar_tensor_tensor(
        tq, in0=xw, scalar=float(sh2), in1=W_ps[:],
        op0=OP.mult, op1=OP.subtract,
    )
    nc.vector.scalar_tensor_tensor(
        tq, in0=tq, scalar=float(-0.5 * g0 / sc2), in1=yw,
        op0=OP.mult, op1=OP.subtract,
    )
    nc.vector.tensor_tensor(tq, tq, xw, op=OP.mult)

    # merged reduce; the ones column carries gamma0 so that
    # g0 * (tq~ | gl~) sums to -0.5*quad - (T/2N)*sum(log f)
    red = wpool.tile([P, 1], f32, tag=f"red{ci}")
    nc.vector.tensor_reduce(red[:], t0[:], axis=mybir.AxisListType.X, op=OP.add)
    ones = cpool.tile([P, 1], f32, tag=f"ones{ci}")
    nc.vector.memset(ones[:], float(g0))
    out_ps = ppool.tile([1, 1], f32, tag="out_ps")
    nc.tensor.matmul(
        out_ps[:], ones[:], red[:], start=True, stop=True,
        skip_group_check=True,
    )
    fin = wpool.tile([1, 1], f32, tag=f"fin{ci}")
    nc.vector.tensor_scalar(
        fin[:], out_ps[:], 1.0, float(-0.5 * corr), op0=OP.mult, op1=OP.add
    )
    nc.sync.dma_start(out_dram[:, ci : ci + 1], fin[:])
